# revision 35
# baseline (speedup 1.0000x reference)
"""NT-Xent loss (SimCLR) on 8 Trainium2 NeuronCores — symmetric-triangle
version.

The 8192x8192 sim matrix is symmetric: each unordered 128x128 tile pair
is computed exactly once across the 8 cores (brute-force-verified cyclic
cover: core c owns tile-rows {8k+c}, row 8k+c covers column tiles
(8k+c+j) mod 64 for j < L_k, L = 33 (k<4) / 32 (k>=4); 260 blocks per
core). SPMD-compatible: every core runs the identical program on an
input whose tiles are rotated by c so its rows sit at positions {8k}.

Each exp block feeds two reductions:
  - row sums on the ACT engine (fused exp+accumulate) — ACT is the
    bottleneck engine, and halving its element count is the point;
  - column sums: the DVE accumulates exp tiles into an SBUF bf16
    accumulator (per-partition partial sums), and one final PE pass
    with a ones-vector as weights reduces over partitions, written to
    HBM straight from PSUM.

Host does the cheap O(N) ends in float64: normalize + bf16-cast the
projections (0.1% of FLOPs), and the final cross-core row-sum
combination + log (needs all-core data anyway).

Positives come for free: with the (p n) d -> p n d layout, row r and
partner r+B share a tile at partitions p and p+64, so positive logits
are the shifted diagonal of each diagonal sim block — extracted by a
DVE mask-multiply-accumulate against Q with Q[p, (p+64)%128] = 2
(the 2 = 1/temperature).
"""

import os
import sys

if "/opt/trn_rl_repo" not in sys.path:
    sys.path.insert(0, "/opt/trn_rl_repo")

import numpy as np

import concourse.bacc as bacc
import concourse.mybir as mybir
import concourse.tile as tile
from concourse.bass_utils import run_bass_kernel_spmd

B = 4096
D = 128
N = 2 * B
CORES = 8
NT = 64  # 128-row tiles
ROWS = 8  # block-rows per core; row k sits at position-tile 8k
SEG = 1536  # main-loop segment width (3 PSUM banks)
EXP2 = float(np.exp(2.0))

f32 = mybir.dt.float32
bf16 = mybir.dt.bfloat16

PREP_ORDER = [4, 5, 6, 7, 0, 1, 2, 3]
ROW_ORDER = [4, 5, 6, 7, 0, 1, 2, 3]


def row_L(k):
    return 33 if k < 4 else 32


def row_ranges(k):
    lo = 1024 * k
    hi = lo + 128 * row_L(k)
    if hi <= 8192:
        return [(lo, hi)]
    return [(lo, 8192), (0, hi - 8192)]


def row_segments(k):
    segs = []
    for a, b in row_ranges(k):
        x = a
        while x < b:
            e = min(x + SEG, b)
            segs.append((x, e))
            x = e
    return segs


def build_schedule():
    """Interleave chunk prep with main-loop segments as data becomes
    ready. Preps 0-3 are spread between early segments (their DMAs have
    landed by then) so their PE transposes slot into gaps instead of
    bunching up and starving the ACT pipeline.
    Events: ("prep", g) | ("seg", k, si, a, b)."""
    remaining = {k: list(enumerate(row_segments(k))) for k in range(ROWS)}
    done = set()
    events = []
    segs_since_prep = [0]

    def emit_ready():
        n = 0
        for k in ROW_ORDER:
            rem = remaining[k]
            while rem and ready(k, rem[0][1][0], rem[0][1][1]):
                si, (a, b) = rem.pop(0)
                events.append(("seg", k, si, a, b))
                n += 1
        return n

    def ready(k, a, b):
        tiles = set(range(a // 128, (b + 127) // 128)) | {8 * k}
        return all((t // 8) in done for t in tiles)

    for g in [4, 5, 6, 7]:
        events.append(("prep", g))
        done.add(g)
    # collect the segments unlocked by chunks 4-7, then interleave the
    # remaining preps after every 2 of them
    pending = []
    for k in ROW_ORDER:
        rem = remaining[k]
        while rem and ready(k, rem[0][1][0], rem[0][1][1]):
            si, (a, b) = rem.pop(0)
            pending.append(("seg", k, si, a, b))
    next_prep = [0, 1, 2, 3]
    for i, ev in enumerate(pending):
        events.append(ev)
        if i % 2 == 1 and next_prep:
            g = next_prep.pop(0)
            events.append(("prep", g))
            done.add(g)
    while next_prep:
        g = next_prep.pop(0)
        events.append(("prep", g))
        done.add(g)
    emit_ready()
    assert all(not v for v in remaining.values())
    return events


SCHEDULE = build_schedule()
NSEG = sum(1 for e in SCHEDULE if e[0] == "seg")
assert NSEG == 25, NSEG

# For each 512-col window of the exp accumulator, the index (in seg
# emission order) of the last segment whose accumulate touches it —
# its DMA to HBM can fire right after that segment.
_LAST_TOUCH = [-1] * 16
_si = 0
for _ev in SCHEDULE:
    if _ev[0] != "seg":
        continue
    _, _k, _s, _a, _b = _ev
    lo = _a + 128 if _s == 0 else _a
    for _w in range(lo // 512, (_b + 511) // 512):
        _LAST_TOUCH[_w] = _si
    _si += 1
assert all(t >= 0 for t in _LAST_TOUCH)


def build_nc():
    nc = bacc.Bacc("TRN2", target_bir_lowering=False, debug=False, num_devices=CORES)
    zh = nc.dram_tensor("zh", [N, D], bf16, kind="ExternalInput").ap()
    eyeb = nc.dram_tensor("eyeb", [128, 128], bf16, kind="ExternalInput").ap()
    qmask = nc.dram_tensor("qmask", [128, 128], f32, kind="ExternalInput").ap()
    out_rp = nc.dram_tensor("rowparts", [128, 32], f32, kind="ExternalOutput").ap()
    out_pos = nc.dram_tensor("posbuf", [128, 8], f32, kind="ExternalOutput").ap()
    out_cs = nc.dram_tensor("colsum", [128, N], bf16, kind="ExternalOutput").ap()

    AF = mybir.ActivationFunctionType
    OP = mybir.AluOpType

    with tile.TileContext(nc) as tc:
        with (
            tc.tile_pool(name="big", bufs=1) as big,
            tc.tile_pool(name="stats", bufs=1) as stats,
            tc.tile_pool(name="ebuf", bufs=5) as ebufp,
            tc.tile_pool(name="mm_ps", bufs=2, space="PSUM") as mm_pool,
            tc.tile_pool(name="fin_ps", bufs=2, space="PSUM") as fin_pool,
        ):
            zhb = big.tile([128, N], bf16, tag="zhb")  # normalized z (p n) d
            zhatT = big.tile([128, N], bf16, tag="zhatT")  # transposed
            eacc = big.tile([128, N], bf16, tag="eacc")  # exp col partials
            eye_b = stats.tile([128, 128], bf16, tag="eye_b")
            qmask_t = stats.tile([128, 128], f32, tag="qmask")
            rowparts = stats.tile([128, 32], f32, tag="rowparts")
            posbuf = stats.tile([128, 8], f32, tag="posbuf")
            pos_scr = stats.tile([128, 128], f32, tag="pos_scr")

            # constants + accumulator init
            nc.sync.dma_start(eye_b[:], eyeb[:])
            # zero the accumulator in prep order so the first segments'
            # accumulates aren't gated on one monolithic memset
            for g in PREP_ORDER:
                nc.gpsimd.memset(eacc[:, g * 1024 : (g + 1) * 1024], 0.0)

            # all chunk loads up front, in prep order; 2D view so each
            # partition's 8 consecutive rows coalesce into one 2KB
            # descriptor instead of 8x512B
            zv = zh.rearrange("(p n) d -> p (n d)", p=128)
            for i, g in enumerate(PREP_ORDER):
                sl = slice(g * 1024, (g + 1) * 1024)
                nc.sync.dma_start(zhb[:, sl], zv[:, sl])
                if i == 1:
                    # qmask is first needed by the pos extraction of the
                    # first segment — well after the first two chunks
                    nc.sync.dma_start(qmask_t[:], qmask[:])

            callmap = []
            seg_idx = 0

            def fire_ready_windows(j):
                # merged DMA of accumulator windows finalized by seg j
                ws = [w for w in range(16) if _LAST_TOUCH[w] == j]
                while ws:
                    w0 = ws.pop(0)
                    w1 = w0
                    while ws and ws[0] == w1 + 1:
                        w1 = ws.pop(0)
                    nc.sync.dma_start(
                        out_cs[:, 512 * w0 : 512 * (w1 + 1)],
                        eacc[:, 512 * w0 : 512 * (w1 + 1)],
                    )

            for ev in SCHEDULE:
                if ev[0] == "prep":
                    g = ev[1]
                    sl = slice(g * 1024, (g + 1) * 1024)
                    pp = fin_pool.tile([128, 512], f32, tag="pp")
                    ppb = pp[:].bitcast(bf16)[:, 0:1024]
                    for t in range(8):
                        nc.tensor.transpose(
                            ppb[:, t * 128 : (t + 1) * 128],
                            zhb[:, (g * 8 + t) * 128 : (g * 8 + t + 1) * 128],
                            eye_b[:],
                        )
                    nc.vector.tensor_copy(zhatT[:, sl], ppb)
                else:
                    _, k, si, a, b = ev
                    w = b - a
                    ps = mm_pool.tile([128, SEG], f32, tag="mm")
                    lhs = zhatT[:, 8 * k * 128 : (8 * k + 1) * 128]
                    off = 0
                    while off < w:
                        ww = min(512, w - off)
                        nc.tensor.matmul(
                            ps[:, off : off + ww],
                            lhsT=lhs,
                            rhs=zhatT[:, a + off : a + off + ww],
                            start=True,
                            stop=True,
                        )
                        off += ww
                    eb = ebufp.tile([128, SEG], bf16, tag="eb")
                    ci = len(callmap)
                    nc.scalar.activation(
                        eb[:, 0:w],
                        ps[:, 0:w],
                        AF.Exp,
                        bias=0.0,
                        scale=2.0,
                        accum_out=rowparts[:, ci : ci + 1],
                    )
                    callmap.append((k, si))
                    if si == 0:
                        nc.vector.scalar_tensor_tensor(
                            pos_scr[:],
                            ps[:, 0:128],
                            1.0,
                            qmask_t[:],
                            OP.mult,
                            OP.mult,
                            accum_out=posbuf[:, k : k + 1],
                        )
                        # exclude the diagonal block from the column-sum
                        # accumulator (its columns are fully covered by the
                        # ACT row pass of this same block)
                        if w > 128:
                            nc.vector.tensor_tensor(
                                eacc[:, a + 128 : b],
                                eacc[:, a + 128 : b],
                                eb[:, 128 : w],
                                op=OP.add,
                            )
                    else:
                        nc.vector.tensor_tensor(
                            eacc[:, a : b],
                            eacc[:, a : b],
                            eb[:, 0 : w],
                            op=OP.add,
                        )
                    fire_ready_windows(seg_idx)
                    seg_idx += 1

            nc.sync.dma_start(out_rp[:], rowparts[:])
            nc.sync.dma_start(out_pos[:], posbuf[:])

    nc.compile()
    return nc, callmap


_NC_CACHE = {}


def _get_nc():
    if "nc" not in _NC_CACHE:
        _NC_CACHE["nc"] = build_nc()
    return _NC_CACHE["nc"]


def kernel(z_i, z_j):
    import jax.numpy as jnp

    z_i = np.asarray(z_i, dtype=np.float64)
    z_j = np.asarray(z_j, dtype=np.float64)
    z = np.concatenate([z_i, z_j], axis=0)
    norm = np.sqrt((z * z).sum(1, keepdims=True))
    zh = (z / np.maximum(norm, 1e-12)).astype(np.float32)
    zh16 = np.asarray(jnp.asarray(zh).astype(jnp.bfloat16))

    eyeb = np.asarray(jnp.eye(128, dtype=jnp.bfloat16))
    qm = np.zeros((128, 128), dtype=np.float32)
    qm[np.arange(128), (np.arange(128) + 64) % 128] = 2.0

    r = np.arange(N)
    in_maps = []
    for c in range(CORES):
        idx = (r // NT) * NT + ((r % NT) + c) % NT
        in_maps.append(
            {"zh": np.ascontiguousarray(zh16[idx]), "eyeb": eyeb, "qmask": qm}
        )

    nc, callmap = _get_nc()
    kwargs = {}
    tdir = os.environ.get("NTX_TRACE_DIR")
    if tdir:
        kwargs = {"trace": True, "tmpdir": tdir, "trace_cores": [0]}
    res = run_bass_kernel_spmd(nc, in_maps, core_ids=list(range(CORES)), **kwargs)
    if tdir:
        _NC_CACHE["last_results"] = res

    rs = np.zeros(N, dtype=np.float64)
    pos = np.zeros(N, dtype=np.float64)
    q = np.arange(N)
    for c in range(CORES):
        xs, ps_ = q // 128, q % 128
        orig = ps_ * NT + ((xs + c) % NT)  # position q -> original row
        rp = res.results[c]["rowparts"].astype(np.float64)
        for ci, (k, si) in enumerate(callmap):
            qpos = 128 * (8 * k) + np.arange(128)
            rs[orig[qpos]] += rp[:, ci]
        cs = res.results[c]["colsum"]  # [128, N] bf16 partial sums
        csflat = np.asarray(cs).astype(np.float64).sum(axis=0)
        rs[orig] += csflat
        pb = res.results[c]["posbuf"].astype(np.float64)
        for k in range(ROWS):
            qpos = 128 * (8 * k) + np.arange(128)
            pos[orig[qpos]] = pb[:, k]

    rs -= EXP2  # diagonal contributes exp(2*||zhat||^2) ~= e^2 per row
    loss = float(np.mean(np.log(rs) - pos))
    return np.float32(loss)


# revision 36
# speedup vs baseline: 1.1383x; 1.1383x over previous
"""NT-Xent loss (SimCLR) on 8 Trainium2 NeuronCores — symmetric-triangle
version.

The 8192x8192 sim matrix is symmetric: each unordered 128x128 tile pair
is computed exactly once across the 8 cores (brute-force-verified cyclic
cover: core c owns tile-rows {8k+c}, row 8k+c covers column tiles
(8k+c+j) mod 64 for j < L_k, L = 33 (k<4) / 32 (k>=4); 260 blocks per
core). SPMD-compatible: every core runs the identical program on an
input whose tiles are rotated by c so its rows sit at positions {8k}.

Each exp block feeds two reductions:
  - row sums on the ACT engine (fused exp+accumulate) — ACT is the
    bottleneck engine, and halving its element count is the point;
  - column sums: the DVE accumulates exp tiles into an SBUF bf16
    accumulator (per-partition partial sums), and one final PE pass
    with a ones-vector as weights reduces over partitions, written to
    HBM straight from PSUM.

Host does the cheap O(N) ends in float64: normalize + bf16-cast the
projections (0.1% of FLOPs), and the final cross-core row-sum
combination + log (needs all-core data anyway).

Positives come for free: with the (p n) d -> p n d layout, row r and
partner r+B share a tile at partitions p and p+64, so positive logits
are the shifted diagonal of each diagonal sim block — extracted by a
DVE mask-multiply-accumulate against Q with Q[p, (p+64)%128] = 2
(the 2 = 1/temperature).
"""

import os
import sys

if "/opt/trn_rl_repo" not in sys.path:
    sys.path.insert(0, "/opt/trn_rl_repo")

import numpy as np

import concourse.bacc as bacc
import concourse.mybir as mybir
import concourse.tile as tile
from concourse.bass_utils import run_bass_kernel_spmd

B = 4096
D = 128
N = 2 * B
CORES = 8
NT = 64  # 128-row tiles
ROWS = 8  # block-rows per core; row k sits at position-tile 8k
SEG = 1536  # main-loop segment width (3 PSUM banks)
EXP2 = float(np.exp(2.0))

f32 = mybir.dt.float32
bf16 = mybir.dt.bfloat16

PREP_ORDER = [4, 5, 6, 7, 0, 1, 2, 3]
ROW_ORDER = [4, 5, 6, 7, 0, 1, 2, 3]


def row_L(k):
    return 33 if k < 4 else 32


def row_ranges(k):
    lo = 1024 * k
    hi = lo + 128 * row_L(k)
    if hi <= 8192:
        return [(lo, hi)]
    return [(lo, 8192), (0, hi - 8192)]


def row_segments(k):
    segs = []
    for a, b in row_ranges(k):
        x = a
        while x < b:
            e = min(x + SEG, b)
            segs.append((x, e))
            x = e
    return segs


def build_schedule():
    """Interleave chunk prep with main-loop segments as data becomes
    ready. Events: ("prep", g) | ("seg", k, si, a, b)."""
    remaining = {k: list(enumerate(row_segments(k))) for k in range(ROWS)}
    done = set()
    events = []

    def ready(k, a, b):
        tiles = set(range(a // 128, (b + 127) // 128)) | {8 * k}
        return all((t // 8) in done for t in tiles)

    for g in PREP_ORDER:
        events.append(("prep", g))
        done.add(g)
        for k in ROW_ORDER:
            rem = remaining[k]
            while rem and ready(k, rem[0][1][0], rem[0][1][1]):
                si, (a, b) = rem.pop(0)
                events.append(("seg", k, si, a, b))
    assert all(not v for v in remaining.values())
    return events


SCHEDULE = build_schedule()
NSEG = sum(1 for e in SCHEDULE if e[0] == "seg")
assert NSEG == 25, NSEG

# For each 512-col window of the exp accumulator, the index (in seg
# emission order) of the last segment whose accumulate touches it —
# its DMA to HBM can fire right after that segment.
_LAST_TOUCH = [-1] * 16
_si = 0
for _ev in SCHEDULE:
    if _ev[0] != "seg":
        continue
    _, _k, _s, _a, _b = _ev
    lo = _a + 128 if _s == 0 else _a
    for _w in range(lo // 512, (_b + 511) // 512):
        _LAST_TOUCH[_w] = _si
    _si += 1
assert all(t >= 0 for t in _LAST_TOUCH)


def build_nc():
    nc = bacc.Bacc("TRN2", target_bir_lowering=False, debug=False, num_devices=CORES)
    zh = nc.dram_tensor("zh", [N, D], bf16, kind="ExternalInput").ap()
    eyeb = nc.dram_tensor("eyeb", [128, 128], bf16, kind="ExternalInput").ap()
    qmask = nc.dram_tensor("qmask", [128, 128], f32, kind="ExternalInput").ap()
    out_rp = nc.dram_tensor("rowparts", [128, 32], f32, kind="ExternalOutput").ap()
    out_pos = nc.dram_tensor("posbuf", [128, 8], f32, kind="ExternalOutput").ap()
    out_cs = nc.dram_tensor("colsum", [128, N], bf16, kind="ExternalOutput").ap()

    AF = mybir.ActivationFunctionType
    OP = mybir.AluOpType

    with tile.TileContext(nc) as tc:
        with (
            tc.tile_pool(name="big", bufs=1) as big,
            tc.tile_pool(name="stats", bufs=1) as stats,
            tc.tile_pool(name="ebuf", bufs=5) as ebufp,
            tc.tile_pool(name="mm_ps", bufs=2, space="PSUM") as mm_pool,
            tc.tile_pool(name="fin_ps", bufs=2, space="PSUM") as fin_pool,
        ):
            zhb = big.tile([128, N], bf16, tag="zhb")  # normalized z (p n) d
            zhatT = big.tile([128, N], bf16, tag="zhatT")  # transposed
            eacc = big.tile([128, N], bf16, tag="eacc")  # exp col partials
            eye_b = stats.tile([128, 128], bf16, tag="eye_b")
            qmask_t = stats.tile([128, 128], f32, tag="qmask")
            rowparts = stats.tile([128, 32], f32, tag="rowparts")
            posbuf = stats.tile([128, 8], f32, tag="posbuf")
            pos_scr = stats.tile([128, 128], f32, tag="pos_scr")

            # constants + accumulator init
            nc.sync.dma_start(eye_b[:], eyeb[:])
            # zero the accumulator in prep order so the first segments'
            # accumulates aren't gated on one monolithic memset
            for g in PREP_ORDER:
                nc.gpsimd.memset(eacc[:, g * 1024 : (g + 1) * 1024], 0.0)

            # all chunk loads up front, in prep order; 2D view so each
            # partition's 8 consecutive rows coalesce into one 2KB
            # descriptor instead of 8x512B
            zv = zh.rearrange("(p n) d -> p (n d)", p=128)
            for i, g in enumerate(PREP_ORDER):
                sl = slice(g * 1024, (g + 1) * 1024)
                nc.sync.dma_start(zhb[:, sl], zv[:, sl])
                if i == 1:
                    # qmask is first needed by the pos extraction of the
                    # first segment — well after the first two chunks
                    nc.sync.dma_start(qmask_t[:], qmask[:])

            callmap = []
            seg_idx = 0

            def fire_ready_windows(j):
                # merged DMA of accumulator windows finalized by seg j
                ws = [w for w in range(16) if _LAST_TOUCH[w] == j]
                while ws:
                    w0 = ws.pop(0)
                    w1 = w0
                    while ws and ws[0] == w1 + 1:
                        w1 = ws.pop(0)
                    nc.sync.dma_start(
                        out_cs[:, 512 * w0 : 512 * (w1 + 1)],
                        eacc[:, 512 * w0 : 512 * (w1 + 1)],
                    )

            for ev in SCHEDULE:
                if ev[0] == "prep":
                    g = ev[1]
                    sl = slice(g * 1024, (g + 1) * 1024)
                    pp = fin_pool.tile([128, 512], f32, tag="pp")
                    ppb = pp[:].bitcast(bf16)[:, 0:1024]
                    for t in range(8):
                        nc.tensor.transpose(
                            ppb[:, t * 128 : (t + 1) * 128],
                            zhb[:, (g * 8 + t) * 128 : (g * 8 + t + 1) * 128],
                            eye_b[:],
                        )
                    nc.vector.tensor_copy(zhatT[:, sl], ppb)
                else:
                    _, k, si, a, b = ev
                    w = b - a
                    ps = mm_pool.tile([128, SEG], f32, tag="mm")
                    lhs = zhatT[:, 8 * k * 128 : (8 * k + 1) * 128]
                    off = 0
                    while off < w:
                        ww = min(512, w - off)
                        nc.tensor.matmul(
                            ps[:, off : off + ww],
                            lhsT=lhs,
                            rhs=zhatT[:, a + off : a + off + ww],
                            start=True,
                            stop=True,
                        )
                        off += ww
                    eb = ebufp.tile([128, SEG], bf16, tag="eb")
                    ci = len(callmap)
                    nc.scalar.activation(
                        eb[:, 0:w],
                        ps[:, 0:w],
                        AF.Exp,
                        bias=0.0,
                        scale=2.0,
                        accum_out=rowparts[:, ci : ci + 1],
                    )
                    callmap.append((k, si))
                    if si == 0:
                        nc.vector.scalar_tensor_tensor(
                            pos_scr[:],
                            ps[:, 0:128],
                            1.0,
                            qmask_t[:],
                            OP.mult,
                            OP.mult,
                            accum_out=posbuf[:, k : k + 1],
                        )
                        # exclude the diagonal block from the column-sum
                        # accumulator (its columns are fully covered by the
                        # ACT row pass of this same block)
                        if w > 128:
                            nc.vector.tensor_tensor(
                                eacc[:, a + 128 : b],
                                eacc[:, a + 128 : b],
                                eb[:, 128 : w],
                                op=OP.add,
                            )
                    else:
                        nc.vector.tensor_tensor(
                            eacc[:, a : b],
                            eacc[:, a : b],
                            eb[:, 0 : w],
                            op=OP.add,
                        )
                    fire_ready_windows(seg_idx)
                    seg_idx += 1

            nc.sync.dma_start(out_rp[:], rowparts[:])
            nc.sync.dma_start(out_pos[:], posbuf[:])

    nc.compile()
    return nc, callmap


_NC_CACHE = {}


def _get_nc():
    if "nc" not in _NC_CACHE:
        _NC_CACHE["nc"] = build_nc()
    return _NC_CACHE["nc"]


def kernel(z_i, z_j):
    import jax.numpy as jnp

    z_i = np.asarray(z_i, dtype=np.float64)
    z_j = np.asarray(z_j, dtype=np.float64)
    z = np.concatenate([z_i, z_j], axis=0)
    norm = np.sqrt((z * z).sum(1, keepdims=True))
    zh = (z / np.maximum(norm, 1e-12)).astype(np.float32)
    zh16 = np.asarray(jnp.asarray(zh).astype(jnp.bfloat16))

    eyeb = np.asarray(jnp.eye(128, dtype=jnp.bfloat16))
    qm = np.zeros((128, 128), dtype=np.float32)
    qm[np.arange(128), (np.arange(128) + 64) % 128] = 2.0

    r = np.arange(N)
    in_maps = []
    for c in range(CORES):
        idx = (r // NT) * NT + ((r % NT) + c) % NT
        in_maps.append(
            {"zh": np.ascontiguousarray(zh16[idx]), "eyeb": eyeb, "qmask": qm}
        )

    nc, callmap = _get_nc()
    kwargs = {}
    tdir = os.environ.get("NTX_TRACE_DIR")
    if tdir:
        kwargs = {"trace": True, "tmpdir": tdir, "trace_cores": [0]}
    res = run_bass_kernel_spmd(nc, in_maps, core_ids=list(range(CORES)), **kwargs)
    if tdir:
        _NC_CACHE["last_results"] = res

    rs = np.zeros(N, dtype=np.float64)
    pos = np.zeros(N, dtype=np.float64)
    q = np.arange(N)
    for c in range(CORES):
        xs, ps_ = q // 128, q % 128
        orig = ps_ * NT + ((xs + c) % NT)  # position q -> original row
        rp = res.results[c]["rowparts"].astype(np.float64)
        for ci, (k, si) in enumerate(callmap):
            qpos = 128 * (8 * k) + np.arange(128)
            rs[orig[qpos]] += rp[:, ci]
        cs = res.results[c]["colsum"]  # [128, N] bf16 partial sums
        csflat = np.asarray(cs).astype(np.float64).sum(axis=0)
        rs[orig] += csflat
        pb = res.results[c]["posbuf"].astype(np.float64)
        for k in range(ROWS):
            qpos = 128 * (8 * k) + np.arange(128)
            pos[orig[qpos]] = pb[:, k]

    rs -= EXP2  # diagonal contributes exp(2*||zhat||^2) ~= e^2 per row
    loss = float(np.mean(np.log(rs) - pos))
    return np.float32(loss)
